# revision 32
# baseline (speedup 1.0000x reference)
"""Trainium2 Bass kernel for the diagonal complex linear recurrence (SSM scan).

Problem: out[t, d] = z_d * out[t-1, d] + x[t, d],  z_d = exp(-exp(size_d) + i*theta_d)
         x: [T=8192, D=2048] f32, out: [T, D] complex64.

Strategy:
  - Shard channels D across 8 cores (256 each), pure model parallelism.
  - Per core, layout [channels(partitions), time(free)].  The complex scan is
    decomposed per time-chunk of length L via a local phase twist:
        v[jL+l] = e^{i*theta*l} * W_j[l]
        W_j[l]  = r * W_j[l-1] + e^{-i*theta*l} * x[jL+l],   r = |z| (real!)
    which splits into two independent REAL first-order scans (re/im) that map
    onto the hardware tensor_tensor_scan instruction.  At chunk boundaries the
    carry is rotated once per channel: K_j = e^{i*theta*L} * W_{j-1}[L-1].
  - Twist/untwist tables (cos/sin of theta*l) are tiny [256, L] constants
    precomputed on host in float64.
"""

import os
import sys

import numpy as np

for _p in ("/opt/trn_rl_repo", "/root/.axon_site/_ro/trn_rl_repo"):
    if os.path.isdir(_p) and _p not in sys.path:
        sys.path.append(_p)

import concourse.bacc as bacc
import concourse.mybir as mybir
from concourse import bass_utils
from concourse.tile import TileContext

T = 8192
D = 2048
NCORES = 8
DS = D // NCORES          # 256 channels per core
G = DS // 128             # partition groups per core (2)
L = 1024                  # twist-chunk length (scan segment)
C = T // L                # chunks
F32 = mybir.dt.float32

_PROGRAM = None


def _build_program():
    """Build + compile the single-core Bass program (same NEFF on all cores).

    Combined-group layout: SBUF tiles are [128, 2L]; partition p carries
    channel p (group 0) in cols [0:L] and channel p+128 (group 1) in cols
    [L:2L].  Twist/untwist ops then cover both groups in one instruction;
    the scans still run on per-group column slices (carry stays per group).
    Tables arrive pre-concatenated from the host.
    """
    nc = bacc.Bacc("TRN2", target_bir_lowering=False)

    W = 2 * L
    xT = nc.dram_tensor("xT", (DS, T), F32, kind="ExternalInput")
    cosl = nc.dram_tensor("cosl", (128, W), F32, kind="ExternalInput")
    sinl = nc.dram_tensor("sinl", (128, W), F32, kind="ExternalInput")
    nsinl = nc.dram_tensor("nsinl", (128, W), F32, kind="ExternalInput")
    rb = nc.dram_tensor("rb", (128, W), F32, kind="ExternalInput")
    bnd = nc.dram_tensor("bnd", (128, 8), F32, kind="ExternalInput")
    eye = nc.dram_tensor("eye", (128, 128), F32, kind="ExternalInput")
    out_re = nc.dram_tensor("out_re", (DS, T), F32, kind="ExternalOutput")
    out_im = nc.dram_tensor("out_im", (DS, T), F32, kind="ExternalOutput")

    mult = mybir.AluOpType.mult
    add = mybir.AluOpType.add
    ident = mybir.ActivationFunctionType.Identity
    MMF = min(512, L)   # per-matmul free dim (one PSUM bank)
    OH = min(1024, L)   # PSUM tile / ACT-copy / out-DMA granularity

    with TileContext(nc) as tc:
        with tc.tile_pool(name="tabs", bufs=1) as tpool, \
             tc.tile_pool(name="work", bufs=2) as pool, \
             tc.tile_pool(name="kpool", bufs=4) as kpool, \
             tc.tile_pool(name="psum", bufs=2, space="PSUM") as ppool:
            L0 = min(256, L // 2)
            xt0 = pool.tile([128, W], F32, name="xt", tag="xt")
            nsin_t = tpool.tile([128, W], F32, name="nsin_t")
            cos_t = tpool.tile([128, W], F32, name="cos_t")
            rb_t = tpool.tile([128, W], F32, name="rb_t")
            sin_t = tpool.tile([128, W], F32, name="sin_t")
            # lead-in: the first-piece columns of both halves, then the rest
            for h in (0, 1):
                hp = slice(h * 128, (h + 1) * 128)
                hs = slice(h * L, h * L + L0)
                nc.sync.dma_start(xt0[:, hs], xT[hp, 0:L0])
                nc.sync.dma_start(nsin_t[:, hs], nsinl[:, hs])
                nc.sync.dma_start(cos_t[:, hs], cosl[:, hs])
                nc.sync.dma_start(rb_t[:, hs], rb[:, hs])
                nc.sync.dma_start(sin_t[:, hs], sinl[:, hs])
            for h in (0, 1):
                hp = slice(h * 128, (h + 1) * 128)
                hs = slice(h * L + L0, (h + 1) * L)
                nc.sync.dma_start(xt0[:, hs], xT[hp, L0:L])
                nc.sync.dma_start(nsin_t[:, hs], nsinl[:, hs])
                nc.sync.dma_start(cos_t[:, hs], cosl[:, hs])
                nc.sync.dma_start(rb_t[:, hs], rb[:, hs])
                nc.sync.dma_start(sin_t[:, hs], sinl[:, hs])
            bnd_t = tpool.tile([128, 8], F32, name="bnd_t")
            nc.sync.dma_start(bnd_t[:], bnd[:])
            eye_t = tpool.tile([128, 128], F32, name="eye_t")
            nc.sync.dma_start(eye_t[:], eye[:])

            pieces = [(0, 0, L0), (0, L0, L)]
            for j in range(1, C - 1):
                pieces.append((j, 0, L))
            pieces += [(C - 1, 0, L // 2), (C - 1, L // 2, 3 * L // 4),
                       (C - 1, 3 * L // 4, L)]

            K = [[None, None], [None, None]]   # [half][re/im]
            cur = None
            for (j, a, b) in pieces:
                ts = slice(j * L + a, j * L + b)
                if a == 0:
                    if j == 0:
                        xt = xt0
                    else:
                        xt = pool.tile([128, W], F32, name="xt", tag="xt")
                        for h in (0, 1):
                            hp = slice(h * 128, (h + 1) * 128)
                            nc.sync.dma_start(
                                xt[:, h * L:(h + 1) * L],
                                xT[hp, j * L:(j + 1) * L])
                    wre = pool.tile([128, W], F32, name="wre", tag="wre")
                    wim = pool.tile([128, W], F32, name="wim", tag="wim")
                    uim = pool.tile([128, W], F32, name="uim", tag="uim")
                    cur = (xt, wre, wim, uim)
                else:
                    xt, wre, wim, uim = cur

                full = (a == 0 and b == L)
                ranges = [(0, W)] if full else \
                    [(h * L + a, h * L + b) for h in (0, 1)]

                # twist (uim first; ure overwrites xt in place)
                for (u, v) in ranges:
                    nc.vector.tensor_mul(uim[:, u:v], xt[:, u:v],
                                         nsin_t[:, u:v])
                    nc.vector.tensor_mul(xt[:, u:v], xt[:, u:v],
                                         cos_t[:, u:v])

                # scans stay per group half (independent carries)
                for h in (0, 1):
                    su, sv = h * L + a, h * L + b
                    if a == 0:
                        init_re = 0.0 if j == 0 else K[h][0][:]
                        init_im = 0.0 if j == 0 else K[h][1][:]
                    else:
                        init_re = wre[:, su - 1:su]
                        init_im = wim[:, su - 1:su]
                    nc.vector.tensor_tensor_scan(
                        wre[:, su:sv], rb_t[:, su:sv], xt[:, su:sv],
                        init_re, op0=mult, op1=add)
                    nc.vector.tensor_tensor_scan(
                        wim[:, su:sv], rb_t[:, su:sv], uim[:, su:sv],
                        init_im, op0=mult, op1=add)

                # boundary carry rotation on ScalarE
                if b == L and j < C - 1:
                    for h in (0, 1):
                        cL = bnd_t[:, 4 * h + 0:4 * h + 1]
                        sL = bnd_t[:, 4 * h + 1:4 * h + 2]
                        nsL = bnd_t[:, 4 * h + 2:4 * h + 3]
                        wreL = wre[:, (h + 1) * L - 1:(h + 1) * L]
                        wimL = wim[:, (h + 1) * L - 1:(h + 1) * L]
                        tmp1 = kpool.tile([128, 1], F32, name="tmp1",
                                          tag="tmp1")
                        tmp2 = kpool.tile([128, 1], F32, name="tmp2",
                                          tag="tmp2")
                        kre = kpool.tile([128, 1], F32, name="kre", tag="kre")
                        kim = kpool.tile([128, 1], F32, name="kim", tag="kim")
                        nc.scalar.activation(tmp1[:], wreL, ident, scale=cL)
                        nc.scalar.activation(kre[:], wimL, ident,
                                             scale=nsL, bias=tmp1[:])
                        nc.scalar.activation(tmp2[:], wreL, ident, scale=sL)
                        nc.scalar.activation(kim[:], wimL, ident,
                                             scale=cL, bias=tmp2[:])
                        K[h][0], K[h][1] = kre, kim

                # untwist products (DVE) + adds (PE->PSUM) + copy (ScalarE)
                for (u, v) in ranges:
                    n = v - u
                    t1 = pool.tile([128, n], F32, name="t1", tag="t1")
                    t2 = pool.tile([128, n], F32, name="t2", tag="t2")
                    nc.vector.tensor_mul(t1[:], cos_t[:, u:v], wre[:, u:v])
                    nc.vector.tensor_mul(t2[:], nsin_t[:, u:v], wim[:, u:v])
                    t3 = pool.tile([128, n], F32, name="t3", tag="t3")
                    t4 = pool.tile([128, n], F32, name="t4", tag="t4")
                    nc.vector.tensor_mul(t3[:], sin_t[:, u:v], wre[:, u:v])
                    nc.vector.tensor_mul(t4[:], cos_t[:, u:v], wim[:, u:v])
                    for o in range(0, n, OH):
                        m = min(OH, n - o)
                        pre = ppool.tile([128, m], F32, name="pre", tag="pre")
                        pim = ppool.tile([128, m], F32, name="pim", tag="pim")
                        for q in range(o, o + m, MMF):
                            qn = min(MMF, o + m - q)
                            qs = slice(q, q + qn)
                            ps = slice(q - o, q - o + qn)
                            nc.tensor.matmul(pre[:, ps], eye_t[:], t1[:, qs],
                                             start=True, stop=False)
                            nc.tensor.matmul(pre[:, ps], eye_t[:], t2[:, qs],
                                             start=False, stop=True)
                            nc.tensor.matmul(pim[:, ps], eye_t[:], t3[:, qs],
                                             start=True, stop=False)
                            nc.tensor.matmul(pim[:, ps], eye_t[:], t4[:, qs],
                                             start=False, stop=True)
                        ore = pool.tile([128, m], F32, name="ore", tag="ore")
                        oim = pool.tile([128, m], F32, name="oim", tag="oim")
                        nc.scalar.copy(ore[:], pre[:])
                        nc.scalar.copy(oim[:], pim[:])
                        # map tile cols back to (group half, time range)
                        cu = u + o
                        h = cu // L
                        hp = slice(h * 128, (h + 1) * 128)
                        tcol = cu - h * L
                        ots = slice(j * L + tcol, j * L + tcol + m)
                        nc.sync.dma_start(out_re[hp, ots], ore[:])
                        nc.sync.dma_start(out_im[hp, ots], oim[:])

    nc.compile()
    return nc


def _get_program():
    global _PROGRAM
    if _PROGRAM is None:
        _PROGRAM = _build_program()
    return _PROGRAM


def _host_prep(x, size, theta):
    """Per-core input maps (host-side sharding + table precompute)."""
    size64 = np.asarray(size, np.float64)
    theta64 = np.asarray(theta, np.float64)
    r64 = np.exp(-np.exp(size64))                      # [D]
    l64 = np.arange(L, dtype=np.float64)
    ang = theta64[:, None] * l64[None, :]              # [D, L]
    cosl = np.cos(ang).astype(np.float32)
    sinl = np.sin(ang).astype(np.float32)
    nsinl = (-np.sin(ang)).astype(np.float32)
    rbf = np.broadcast_to(r64[:, None], (D, L)).astype(np.float32)
    bnd = np.zeros((D, 4), np.float32)
    bnd[:, 0] = np.cos(theta64 * L)
    bnd[:, 1] = np.sin(theta64 * L)
    bnd[:, 2] = -np.sin(theta64 * L)

    x = np.asarray(x, np.float32)
    eye = np.eye(128, dtype=np.float32)

    def comb(t, sl):  # [DS, n] core slice -> combined [128, 2n]
        s = t[sl]
        return np.ascontiguousarray(np.concatenate([s[:128], s[128:]], axis=1))

    in_maps = []
    for c in range(NCORES):
        sl = slice(c * DS, (c + 1) * DS)
        in_maps.append({
            "xT": np.ascontiguousarray(x[:, sl].T),
            "cosl": comb(cosl, sl),
            "sinl": comb(sinl, sl),
            "nsinl": comb(nsinl, sl),
            "rb": comb(rbf, sl),
            "bnd": comb(bnd, sl),
            "eye": eye,
        })
    return in_maps


def _assemble(results):
    out = np.empty((T, D), np.complex64)
    for c, res in enumerate(results):
        sl = slice(c * DS, (c + 1) * DS)
        out[:, sl] = (res["out_re"] + 1j * res["out_im"]).T
    return out


def run(x, size, theta, trace=False, **spmd_kwargs):
    nc = _get_program()
    in_maps = _host_prep(x, size, theta)
    res = bass_utils.run_bass_kernel_spmd(
        nc, in_maps, core_ids=list(range(NCORES)), trace=trace, **spmd_kwargs)
    return _assemble(res.results), res


def kernel(x, size, theta):
    out, _ = run(x, size, theta, trace=False)
    return out
